# revision 14
# baseline (speedup 1.0000x reference)
"""Multi-head attention (B=4, N=2048, DIM=768, H=8, DH=96) on 8 TRN2 NeuronCores.

Sharding: data-parallel over (batch, query-half) — core c handles batch c//2,
query rows [(c%2)*1024, (c%2+1)*1024). Each core computes K/V for its full
batch (duplicated across the 2 cores sharing a batch), so there are NO
collectives: each core produces its own disjoint output shard.

Per-core compute (all matmuls bf16, fp32 PSUM accumulation):
  - Dense Q^T/K^T projection (6 f-tiles of 128), then SBUF->SBUF DMA repack
    (gpsimd queue) into per-head [96, 512] tiles for the attention matmuls.
  - V projection in natural space; a constant 1.0 column is appended per head
    (V|1) so the attn@V matmul also produces the softmax row-sums.
  - dots P^T[nk,nq]: lhsT=K^T[96dh, 128nk], rhs=Q^T[96dh, 512nq]; softmax
    scale folded into w_q host-side. exp() on ScalarE PSUM->SBUF(bf16), no
    max subtraction (logits max ~9, fp32-safe).
  - O'^T[97, nq] accumulated over 16 key tiles; row 96 = row-sum s.
  - normalize (no PE involvement): evacuate O' to SBUF, move s to partition 0
    (gpsimd DMA), gpsimd partition_broadcast, reciprocal_approx_fast (DVE),
    multiply.
  - proj y^T[c,nq]: heads 0-6 accumulated as fillers during the last head's
    attention (bias folded in), head 7 added at the tail.

Emission is software-pipelined: projection/repack chunks are interleaved into
the attention t-loops so ScalarE's exp stream never starves and the PE
always has ready work.

Output per core: y^T [768, 1024] fp32; host transposes/reassembles.
"""

import numpy as np
import ml_dtypes

B, N, DIM = 4, 2048, 768
H, DH = 8, 96
NQ = N // 2        # query rows per core
SCALE = DH ** -0.5
NCORES = 8
CT = DIM // 128    # 6 contraction chunks
FT = DIM // 128    # 6 dense f-tiles for Q/K
NT = N // 128      # 16 key tiles
NQC = NQ // 512    # 2 query chunks of 512
NKC = N // 512     # 4 key chunks of 512

_CACHE = {}


def _head_pieces(h):
    """Dense-tile pieces covering f-rows [96h, 96h+96): (j, src_lo, src_hi, dst_lo)."""
    lo, hi = DH * h, DH * h + DH
    out = []
    for j in range(lo // 128, (hi - 1) // 128 + 1):
        a = max(0, lo - 128 * j)
        b = min(128, hi - 128 * j)
        out.append((j, a, b, 128 * j + a - lo))
    return out


def _build():
    import concourse.mybir as mybir
    import concourse.tile as tile
    from concourse import bacc

    f32 = mybir.dt.float32
    bf16 = mybir.dt.bfloat16
    Exp = mybir.ActivationFunctionType.Exp
    mult = mybir.AluOpType.mult
    add = mybir.AluOpType.add

    nc = bacc.Bacc("TRN2", debug=False, num_devices=NCORES)

    xt_d = nc.dram_tensor("xt", [DIM, N], bf16, kind="ExternalInput")
    wq_d = nc.dram_tensor("wq", [DIM, DIM], bf16, kind="ExternalInput")
    wk_d = nc.dram_tensor("wk", [DIM, DIM], bf16, kind="ExternalInput")
    wv_d = nc.dram_tensor("wv", [DIM, DIM], bf16, kind="ExternalInput")
    wp_d = nc.dram_tensor("wp", [H, DH, DIM], bf16, kind="ExternalInput")
    bias_d = nc.dram_tensor("bias", [DIM, 1], f32, kind="ExternalInput")
    out_d = nc.dram_tensor("out", [DIM, NQ], f32, kind="ExternalOutput")

    with tile.TileContext(nc) as tc:
        with (
            tc.tile_pool(name="const", bufs=1) as cpool,
            tc.tile_pool(name="ptp", bufs=3) as pt_pool,
            tc.tile_pool(name="onp", bufs=16) as on_pool,
            tc.tile_pool(name="smallp", bufs=2) as small_pool,
            tc.tile_pool(name="stagep", bufs=6) as stage_pool,
            tc.tile_pool(name="ysb", bufs=2) as y_pool,
            tc.tile_pool(name="ps_qkv", bufs=2, space="PSUM") as psum_qkv,
            tc.tile_pool(name="ps_d", bufs=4, space="PSUM") as psum_d,
            tc.tile_pool(name="ps_o", bufs=2, space="PSUM") as psum_o,
        ):
            # ---- persistent SBUF tensors, consolidated input DMAs ----
            xt_a = cpool.tile([128, CT, NQ], bf16, name="xt_a")
            xt_b = cpool.tile([128, CT, NQ], bf16, name="xt_b")
            wk_sb = cpool.tile([128, CT, DIM], bf16, name="wk_sb")
            wq_sb = cpool.tile([128, CT, DIM], bf16, name="wq_sb")
            wv_sb = cpool.tile([128, CT, DIM], bf16, name="wv_sb")
            wp_sb = cpool.tile([DH, H, DIM], bf16, name="wp_sb")
            bias_sb = cpool.tile([128, CT, 1], f32, name="bias_sb")
            qt_sb = [
                [cpool.tile([DH, 512], bf16, name=f"qt{h}_{qc}") for qc in range(NQC)]
                for h in range(H)
            ]
            kt_sb = [
                [cpool.tile([DH, 512], bf16, name=f"kt{h}_{nc_}") for nc_ in range(NKC)]
                for h in range(H)
            ]
            v_sb = [cpool.tile([128, H, DH + 1], bf16, name=f"v{t}") for t in range(NT)]
            y1_sb = [
                [cpool.tile([128, 512], bf16, name=f"y1_{ct}_{qc}") for qc in range(NQC)]
                for ct in range(CT)
            ]

            xt_r = xt_d.ap().rearrange("(c p) n -> p c n", p=128)
            nc.sync.dma_start(xt_a[:], xt_r[:, :, 0:NQ])
            nc.sync.dma_start(wk_sb[:], wk_d.ap().rearrange("(c p) f -> p c f", p=128))
            nc.sync.dma_start(wq_sb[:], wq_d.ap().rearrange("(c p) f -> p c f", p=128))
            nc.sync.dma_start(xt_b[:], xt_r[:, :, NQ:N])
            nc.sync.dma_start(wv_sb[:], wv_d.ap().rearrange("(c p) f -> p c f", p=128))
            nc.sync.dma_start(wp_sb[:], wp_d.ap().rearrange("h p n -> p h n"))
            nc.sync.dma_start(
                bias_sb[:], bias_d.ap().rearrange("(c p) o -> p c o", p=128)
            )

            for t in range(NT):
                nc.vector.memset(v_sb[t][:, :, DH:DH + 1], 1.0)

            def xt_cols(lo):
                src = xt_a if lo < NQ else xt_b
                off = lo if lo < NQ else lo - NQ
                return src, off

            # ---- dense Q/K chunks + per-head repack ----
            kd_tiles, qd_tiles = {}, {}

            def kd_chunk(j, nc_):
                src, off = xt_cols(nc_ * 512)
                ps = psum_qkv.tile([128, 512], f32, name="kdps", tag="qkvps")
                for ct in range(CT):
                    nc.tensor.matmul(
                        ps,
                        lhsT=wk_sb[:, ct, j * 128:(j + 1) * 128],
                        rhs=src[:, ct, off:off + 512],
                        start=(ct == 0),
                        stop=(ct == CT - 1),
                    )
                t_ = stage_pool.tile([128, 512], bf16, name="kdst", tag="kdst")
                nc.vector.tensor_copy(out=t_[:], in_=ps[:])
                kd_tiles[(j, nc_)] = t_

            def qd_chunk(j, qc):
                src, off = xt_cols(qc * 512)
                ps = psum_qkv.tile([128, 512], f32, name="qdps", tag="qkvps")
                for ct in range(CT):
                    nc.tensor.matmul(
                        ps,
                        lhsT=wq_sb[:, ct, j * 128:(j + 1) * 128],
                        rhs=src[:, ct, off:off + 512],
                        start=(ct == 0),
                        stop=(ct == CT - 1),
                    )
                t_ = stage_pool.tile([128, 512], bf16, name="qdst", tag="qdst", bufs=4)
                nc.vector.tensor_copy(out=t_[:], in_=ps[:])
                qd_tiles[(j, qc)] = t_

            def k_pack(h, nc_):
                for (j, a, b, d) in _head_pieces(h):
                    if (j, nc_) not in kd_tiles:
                        kd_chunk(j, nc_)
                    nc.gpsimd.dma_start(
                        kt_sb[h][nc_][d:d + b - a, :], kd_tiles[(j, nc_)][a:b, :]
                    )

            def q_pack(h, qc):
                for (j, a, b, d) in _head_pieces(h):
                    if (j, qc) not in qd_tiles:
                        qd_chunk(j, qc)
                    nc.gpsimd.dma_start(
                        qt_sb[h][qc][d:d + b - a, :], qd_tiles[(j, qc)][a:b, :]
                    )

            def v_chunk(t, fc):
                src, off = xt_cols(t * 128)
                ps = psum_qkv.tile([128, 512], f32, name="vps", tag="qkvps")
                vps = ps[:, :4 * DH]
                for ct in range(CT):
                    nc.tensor.matmul(
                        vps,
                        lhsT=src[:, ct, off:off + 128],
                        rhs=wv_sb[:, ct, fc * 4 * DH:(fc + 1) * 4 * DH],
                        start=(ct == 0),
                        stop=(ct == CT - 1),
                    )
                for j in range(4):
                    nc.vector.tensor_copy(
                        out=v_sb[t][:, fc * 4 + j, 0:DH],
                        in_=ps[:, j * DH:(j + 1) * DH],
                    )

            on_sb = {}
            y2p = {}

            def proj06(ct, qc):
                """Accumulate heads 0-6 of the projection; bias folded in."""
                yp = psum_qkv.tile([128, 512], f32, name="yps", tag="qkvps")
                for h in range(7):
                    nc.tensor.matmul(
                        yp,
                        lhsT=wp_sb[:, h, ct * 128:(ct + 1) * 128],
                        rhs=on_sb[(h, qc)][:],
                        start=(h == 0),
                        stop=(h == 6),
                    )
                nc.vector.tensor_scalar_add(y1_sb[ct][qc][:], yp[:], bias_sb[:, ct, :])

            def proj7(ct, qc):
                yp = psum_d.tile([128, 512], f32, name="yp7", tag="dps")
                nc.tensor.matmul(
                    yp,
                    lhsT=wp_sb[:, 7, ct * 128:(ct + 1) * 128],
                    rhs=on_sb[(7, qc)][:],
                    start=True,
                    stop=True,
                )
                y_sb = y_pool.tile([128, 512], f32, name="y", tag="y")
                nc.vector.tensor_tensor(y_sb[:], yp[:], y1_sb[ct][qc][:], add)
                nc.sync.dma_start(
                    out_d.ap()[ct * 128:(ct + 1) * 128, qc * 512:(qc + 1) * 512],
                    y_sb[:],
                )

            # ---- attention for one head, with fillers interleaved per slot ----
            def attn_head(h, fillers):
                o_ps = [
                    psum_o.tile([DH + 1, 512], f32, name=f"ops{qc}", tag="ops")
                    for qc in range(NQC)
                ]
                for t in range(NT):
                    for qc in range(NQC):
                        d_ps = psum_d.tile([128, 512], f32, name="dps", tag="dps")
                        nc.tensor.matmul(
                            d_ps[:],
                            lhsT=kt_sb[h][t // 4][:, (t % 4) * 128:(t % 4 + 1) * 128],
                            rhs=qt_sb[h][qc][:],
                            start=True,
                            stop=True,
                        )
                        pt = pt_pool.tile([128, 512], bf16, name="pt", tag="pt")
                        nc.scalar.activation(pt[:], d_ps[:], Exp)
                        nc.tensor.matmul(
                            o_ps[qc],
                            lhsT=v_sb[t][:, h, :],
                            rhs=pt[:],
                            start=(t == 0),
                            stop=(t == NT - 1),
                        )
                    for fn in fillers.get(t, ()):
                        fn()
                # normalize — no PE involvement, all off the critical path
                for qc in range(NQC):
                    o_st = small_pool.tile(
                        [DH + 1, 512], f32, name="ostage", tag="ostage", bufs=3
                    )
                    nc.vector.tensor_copy(out=o_st[:], in_=o_ps[qc][:])
                    s0 = small_pool.tile([1, 512], f32, name="s0", tag="s0")
                    nc.gpsimd.dma_start(s0[:], o_st[DH:DH + 1, :])
                    sb = small_pool.tile([DH, 512], f32, name="sbc", tag="sbc")
                    nc.gpsimd.partition_broadcast(sb[:], s0[:])
                    nc.vector.reciprocal_approx_fast(out=sb[:], in_=sb[:])
                    on = on_pool.tile([DH, 512], bf16, name="on", tag="on")
                    on_sb[(h, qc)] = on
                    nc.vector.tensor_tensor(on[:], o_st[0:DH, :], sb[:], mult)

            # ---- software-pipelined emission ----
            k_pack(0, 0)
            q_pack(0, 0)
            q_pack(0, 1)
            v_chunk(0, 0)
            v_chunk(1, 0)

            def mk_fillers(h):
                f = {}

                def addf(slot, fn):
                    f.setdefault(slot, []).append(fn)

                if h == 0:
                    # own remaining K chunks just-in-time + V tiles JIT
                    for nc_ in (1, 2, 3):
                        addf(4 * nc_ - 3, (lambda n=nc_: k_pack(0, n)))
                    for t in range(2, NT):
                        addf(t - 2, lambda tt=t: v_chunk(tt, 0))
                elif h in (1, 2, 3):
                    start = [0, 6, 11][h - 1]
                    end = [6, 11, 16][h - 1]
                    slots = (1, 3, 5, 7, 9, 11)
                    for i, t in enumerate(range(start, end)):
                        addf(slots[i], lambda tt=t: v_chunk(tt, 1))
                # next head's projections
                if h + 1 < H:
                    for i, nc_ in enumerate((0, 1, 2, 3)):
                        addf(2 * i + 2, (lambda hh=h + 1, n=nc_: k_pack(hh, n)))
                    addf(11, lambda hh=h + 1: q_pack(hh, 0))
                    addf(13, lambda hh=h + 1: q_pack(hh, 1))
                # heads 0-6 projection during the last head's attention
                if h == 7:
                    for i in range(12):
                        ct, qc = i // 2, i % 2
                        addf(i + 1, lambda c=ct, q=qc: proj06(c, q))
                return f

            for h in range(H):
                attn_head(h, mk_fillers(h))

            # ---- tail: head-7 projection + combine + out ----
            for ct in range(CT):
                for qc in range(NQC):
                    proj7(ct, qc)

    nc.compile()
    return nc


def _get_nc():
    if "nc" not in _CACHE:
        _CACHE["nc"] = _build()
    return _CACHE["nc"]


def _prep_shards(x, w_qkv, w_proj, b_proj):
    bf16 = ml_dtypes.bfloat16
    x = np.asarray(x, dtype=np.float32)
    w_qkv = np.asarray(w_qkv, dtype=np.float32)
    w_proj = np.asarray(w_proj, dtype=np.float32)
    b_proj = np.asarray(b_proj, dtype=np.float32)

    wq_b = np.ascontiguousarray(w_qkv[0:DIM].T * SCALE).astype(bf16)
    wk_b = np.ascontiguousarray(w_qkv[DIM:2 * DIM].T).astype(bf16)
    wv_b = np.ascontiguousarray(w_qkv[2 * DIM:3 * DIM].T).astype(bf16)
    wp_b = np.ascontiguousarray(w_proj.T.reshape(H, DH, DIM)).astype(bf16)
    bias = np.ascontiguousarray(b_proj.reshape(DIM, 1))

    in_maps = []
    for c in range(NCORES):
        b, half = divmod(c, 2)
        xt = x[b].T  # [768, 2048]
        if half == 1:
            xt = np.concatenate([xt[:, NQ:], xt[:, :NQ]], axis=1)
        in_maps.append({
            "xt": np.ascontiguousarray(xt).astype(bf16),
            "wq": wq_b,
            "wk": wk_b,
            "wv": wv_b,
            "wp": wp_b,
            "bias": bias,
        })
    return in_maps


def kernel(x, w_qkv, w_proj, b_proj):
    from concourse.bass_utils import run_bass_kernel_spmd

    nc = _get_nc()
    in_maps = _prep_shards(x, w_qkv, w_proj, b_proj)
    res = run_bass_kernel_spmd(nc, in_maps, core_ids=list(range(NCORES)))
    out = np.empty((B, N, DIM), np.float32)
    for c in range(NCORES):
        b, half = divmod(c, 2)
        yT = np.asarray(res.results[c]["out"], dtype=np.float32)  # [768, 1024]
        out[b, half * NQ:(half + 1) * NQ, :] = yT.T
    return out


# revision 19
# speedup vs baseline: 1.0446x; 1.0446x over previous
"""Multi-head attention (B=4, N=2048, DIM=768, H=8, DH=96) on 8 TRN2 NeuronCores.

Sharding: data-parallel over (batch, query-half) — core c handles batch c//2,
query rows [(c%2)*1024, (c%2+1)*1024). Each core computes K/V for its full
batch (duplicated across the 2 cores sharing a batch), so there are NO
collectives: each core produces its own disjoint output shard.

Per-core compute (all matmuls bf16, fp32 PSUM accumulation):
  - Dense Q^T/K^T projection (6 f-tiles of 128), then SBUF->SBUF DMA repack
    (gpsimd queue) into per-head [96, 512] tiles for the attention matmuls.
  - V projection in natural space; a constant 1.0 column is appended per head
    (V|1) so the attn@V matmul also produces the softmax row-sums.
  - dots P^T[nk,nq]: lhsT=K^T[96dh, 128nk], rhs=Q^T[96dh, 512nq]; softmax
    scale folded into w_q host-side. exp() on ScalarE PSUM->SBUF(bf16), no
    max subtraction (logits max ~9, fp32-safe).
  - O'^T[97, nq] accumulated over 16 key tiles; row 96 = row-sum s.
  - normalize (no PE involvement): evacuate O' to SBUF, move s to partition 0
    (gpsimd DMA), gpsimd partition_broadcast, reciprocal_approx_fast (DVE),
    multiply.
  - proj y^T[c,nq]: heads 0-6 accumulated as fillers during the last head's
    attention (bias folded in), head 7 added at the tail.

Emission is software-pipelined: projection/repack chunks are interleaved into
the attention t-loops so ScalarE's exp stream never starves and the PE
always has ready work.

Output per core: y^T [768, 1024] fp32; host transposes/reassembles.
"""

import numpy as np
import ml_dtypes

B, N, DIM = 4, 2048, 768
H, DH = 8, 96
NQ = N // 2        # query rows per core
SCALE = DH ** -0.5
NCORES = 8
CT = DIM // 128    # 6 contraction chunks
FT = DIM // 128    # 6 dense f-tiles for Q/K
NT = N // 128      # 16 key tiles
NQC = NQ // 512    # 2 query chunks of 512
NKC = N // 512     # 4 key chunks of 512

_CACHE = {}


def _head_pieces(h):
    """Dense-tile pieces covering f-rows [96h, 96h+96): (j, src_lo, src_hi, dst_lo)."""
    lo, hi = DH * h, DH * h + DH
    out = []
    for j in range(lo // 128, (hi - 1) // 128 + 1):
        a = max(0, lo - 128 * j)
        b = min(128, hi - 128 * j)
        out.append((j, a, b, 128 * j + a - lo))
    return out


def _build():
    import concourse.mybir as mybir
    import concourse.tile as tile
    from concourse import bacc

    f32 = mybir.dt.float32
    bf16 = mybir.dt.bfloat16
    Exp = mybir.ActivationFunctionType.Exp
    mult = mybir.AluOpType.mult
    add = mybir.AluOpType.add

    nc = bacc.Bacc("TRN2", debug=False, num_devices=NCORES)

    xt_d = nc.dram_tensor("xt", [DIM, N], bf16, kind="ExternalInput")
    wq_d = nc.dram_tensor("wq", [DIM, DIM], bf16, kind="ExternalInput")
    wk_d = nc.dram_tensor("wk", [DIM, DIM], bf16, kind="ExternalInput")
    wv_d = nc.dram_tensor("wv", [DIM, DIM], bf16, kind="ExternalInput")
    wp_d = nc.dram_tensor("wp", [H, DH, DIM], bf16, kind="ExternalInput")
    bias_d = nc.dram_tensor("bias", [DIM, 1], f32, kind="ExternalInput")
    out_d = nc.dram_tensor("out", [DIM, NQ], f32, kind="ExternalOutput")

    with tile.TileContext(nc) as tc:
        with (
            tc.tile_pool(name="const", bufs=1) as cpool,
            tc.tile_pool(name="ptp", bufs=3) as pt_pool,
            tc.tile_pool(name="onp", bufs=16) as on_pool,
            tc.tile_pool(name="smallp", bufs=2) as small_pool,
            tc.tile_pool(name="stagep", bufs=6) as stage_pool,
            tc.tile_pool(name="ysb", bufs=4) as y_pool,
            tc.tile_pool(name="ps_qkv", bufs=2, space="PSUM") as psum_qkv,
            tc.tile_pool(name="ps_d", bufs=4, space="PSUM") as psum_d,
            tc.tile_pool(name="ps_o", bufs=2, space="PSUM") as psum_o,
        ):
            # ---- persistent SBUF tensors, consolidated input DMAs ----
            xt_a = cpool.tile([128, CT, NQ], bf16, name="xt_a")
            xt_b = cpool.tile([128, CT, NQ], bf16, name="xt_b")
            wk_sb = cpool.tile([128, CT, DIM], bf16, name="wk_sb")
            wq_sb = cpool.tile([128, CT, DIM], bf16, name="wq_sb")
            wv_sb = cpool.tile([128, CT, DIM], bf16, name="wv_sb")
            wp_sb = cpool.tile([DH, H, DIM], bf16, name="wp_sb")
            bias_sb = cpool.tile([128, CT, 1], f32, name="bias_sb")
            qt_sb = {}   # h -> [DH, NQ] tile (rotating)
            kt_sb = {}   # h -> [DH, N] tile (rotating)
            v_sb = [cpool.tile([128, H, DH + 1], bf16, name=f"v{t}") for t in range(NT)]
            y1_sb = [
                [cpool.tile([128, 512], bf16, name=f"y1_{ct}_{qc}") for qc in range(NQC)]
                for ct in range(CT)
            ]

            xt_r = xt_d.ap().rearrange("(c p) n -> p c n", p=128)
            nc.sync.dma_start(xt_a[:], xt_r[:, :, 0:NQ])
            nc.sync.dma_start(wk_sb[:], wk_d.ap().rearrange("(c p) f -> p c f", p=128))
            nc.sync.dma_start(wq_sb[:], wq_d.ap().rearrange("(c p) f -> p c f", p=128))
            nc.sync.dma_start(xt_b[:], xt_r[:, :, NQ:N])
            nc.sync.dma_start(wv_sb[:], wv_d.ap().rearrange("(c p) f -> p c f", p=128))
            nc.sync.dma_start(wp_sb[:], wp_d.ap().rearrange("h p n -> p h n"))
            nc.sync.dma_start(
                bias_sb[:], bias_d.ap().rearrange("(c p) o -> p c o", p=128)
            )

            for t in range(NT):
                nc.vector.memset(v_sb[t][:, :, DH:DH + 1], 1.0)

            def xt_cols(lo):
                src = xt_a if lo < NQ else xt_b
                off = lo if lo < NQ else lo - NQ
                return src, off

            # ---- dense Q/K chunks staged per j-tile, one-DMA-per-head repack ----
            kd_j, qd_j = {}, {}

            def kd_chunk(j, nc_):
                if j not in kd_j:
                    kd_j[j] = stage_pool.tile(
                        [128, N], bf16, name=f"kdj", tag="kdj", bufs=2
                    )
                src, off = xt_cols(nc_ * 512)
                ps = psum_qkv.tile([128, 512], f32, name="kdps", tag="qkvps")
                for ct in range(CT):
                    nc.tensor.matmul(
                        ps,
                        lhsT=wk_sb[:, ct, j * 128:(j + 1) * 128],
                        rhs=src[:, ct, off:off + 512],
                        start=(ct == 0),
                        stop=(ct == CT - 1),
                    )
                nc.vector.tensor_copy(
                    out=kd_j[j][:, nc_ * 512:(nc_ + 1) * 512], in_=ps[:]
                )

            def qd_chunk(j, qc):
                if j not in qd_j:
                    qd_j[j] = stage_pool.tile(
                        [128, NQ], bf16, name=f"qdj", tag="qdj", bufs=2
                    )
                src, off = xt_cols(qc * 512)
                ps = psum_qkv.tile([128, 512], f32, name="qdps", tag="qkvps")
                for ct in range(CT):
                    nc.tensor.matmul(
                        ps,
                        lhsT=wq_sb[:, ct, j * 128:(j + 1) * 128],
                        rhs=src[:, ct, off:off + 512],
                        start=(ct == 0),
                        stop=(ct == CT - 1),
                    )
                nc.vector.tensor_copy(
                    out=qd_j[j][:, qc * 512:(qc + 1) * 512], in_=ps[:]
                )

            def k_chunks(h):
                for (j, a, b, d) in _head_pieces(h):
                    if j not in kd_j:
                        for nc_ in range(NKC):
                            kd_chunk(j, nc_)

            def k_pack(h):
                kt_sb[h] = stage_pool.tile([DH, N], bf16, name="kt", tag="kt", bufs=3)
                for (j, a, b, d) in _head_pieces(h):
                    nc.sync.dma_start(kt_sb[h][d:d + b - a, :], kd_j[j][a:b, :])

            def q_chunks(h):
                for (j, a, b, d) in _head_pieces(h):
                    if j not in qd_j:
                        for qc in range(NQC):
                            qd_chunk(j, qc)

            def q_pack(h):
                qt_sb[h] = stage_pool.tile([DH, NQ], bf16, name="qt", tag="qt", bufs=3)
                for (j, a, b, d) in _head_pieces(h):
                    nc.sync.dma_start(qt_sb[h][d:d + b - a, :], qd_j[j][a:b, :])

            def v_chunk(t, fc):
                src, off = xt_cols(t * 128)
                ps = psum_qkv.tile([128, 512], f32, name="vps", tag="qkvps")
                vps = ps[:, :4 * DH]
                for ct in range(CT):
                    nc.tensor.matmul(
                        vps,
                        lhsT=src[:, ct, off:off + 128],
                        rhs=wv_sb[:, ct, fc * 4 * DH:(fc + 1) * 4 * DH],
                        start=(ct == 0),
                        stop=(ct == CT - 1),
                    )
                for j in range(4):
                    nc.vector.tensor_copy(
                        out=v_sb[t][:, fc * 4 + j, 0:DH],
                        in_=ps[:, j * DH:(j + 1) * DH],
                    )

            on_sb = {}
            y2p = {}

            def proj06(ct, qc):
                """Accumulate heads 0-6 of the projection; bias folded in."""
                yp = psum_qkv.tile([128, 512], f32, name="yps", tag="qkvps")
                for h in range(7):
                    nc.tensor.matmul(
                        yp,
                        lhsT=wp_sb[:, h, ct * 128:(ct + 1) * 128],
                        rhs=on_sb[(h, qc)][:],
                        start=(h == 0),
                        stop=(h == 6),
                    )
                nc.vector.tensor_scalar_add(y1_sb[ct][qc][:], yp[:], bias_sb[:, ct, :])

            def proj7(ct, qc):
                yp = psum_d.tile([128, 512], f32, name="yp7", tag="dps")
                nc.tensor.matmul(
                    yp,
                    lhsT=wp_sb[:, 7, ct * 128:(ct + 1) * 128],
                    rhs=on_sb[(7, qc)][:],
                    start=True,
                    stop=True,
                )
                y_sb = y_pool.tile([128, 512], f32, name="y", tag="y")
                nc.vector.tensor_tensor(y_sb[:], yp[:], y1_sb[ct][qc][:], add)
                nc.sync.dma_start(
                    out_d.ap()[ct * 128:(ct + 1) * 128, qc * 512:(qc + 1) * 512],
                    y_sb[:],
                )

            # ---- attention for one head, with fillers interleaved per slot ----
            def attn_head(h, fillers):
                o_ps = [
                    psum_o.tile([DH + 1, 512], f32, name=f"ops{qc}", tag="ops")
                    for qc in range(NQC)
                ]
                for t in range(NT):
                    for qc in range(NQC):
                        d_ps = psum_d.tile([128, 512], f32, name="dps", tag="dps")
                        nc.tensor.matmul(
                            d_ps[:],
                            lhsT=kt_sb[h][:, t * 128:(t + 1) * 128],
                            rhs=qt_sb[h][:, qc * 512:(qc + 1) * 512],
                            start=True,
                            stop=True,
                        )
                        pt = pt_pool.tile([128, 512], bf16, name="pt", tag="pt")
                        nc.scalar.activation(pt[:], d_ps[:], Exp)
                        nc.tensor.matmul(
                            o_ps[qc],
                            lhsT=v_sb[t][:, h, :],
                            rhs=pt[:],
                            start=(t == 0),
                            stop=(t == NT - 1),
                        )
                    for fn in fillers.get(t, ()):
                        fn()
                # normalize — no PE involvement, all off the critical path
                for qc in range(NQC):
                    o_st = small_pool.tile(
                        [DH + 1, 512], f32, name="ostage", tag="ostage", bufs=3
                    )
                    nc.vector.tensor_copy(out=o_st[:], in_=o_ps[qc][:])
                    s0 = small_pool.tile([1, 512], f32, name="s0", tag="s0")
                    nc.gpsimd.dma_start(s0[:], o_st[DH:DH + 1, :])
                    sb = small_pool.tile([DH, 512], f32, name="sbc", tag="sbc")
                    nc.gpsimd.partition_broadcast(sb[:], s0[:])
                    nc.vector.reciprocal_approx_fast(out=sb[:], in_=sb[:])
                    on = on_pool.tile([DH, 512], bf16, name="on", tag="on")
                    on_sb[(h, qc)] = on
                    nc.vector.tensor_tensor(on[:], o_st[0:DH, :], sb[:], mult)

            # ---- software-pipelined emission ----
            k_chunks(0)
            k_pack(0)
            q_chunks(0)
            q_pack(0)
            v_chunk(0, 0)
            v_chunk(1, 0)

            def mk_fillers(h):
                f = {}

                def addf(slot, fn):
                    f.setdefault(slot, []).append(fn)

                if h == 0:
                    for t in range(2, NT):
                        addf(t - 2, lambda tt=t: v_chunk(tt, 0))
                elif h in (1, 2, 3):
                    start = [0, 6, 11][h - 1]
                    end = [6, 11, 16][h - 1]
                    slots = (1, 3, 5, 7, 9, 11)
                    for i, t in enumerate(range(start, end)):
                        addf(slots[i], lambda tt=t: v_chunk(tt, 1))
                # next head's projections: new dense chunks, then repacks
                if h + 1 < H:
                    addf(2, lambda hh=h + 1: k_chunks(hh))
                    addf(6, lambda hh=h + 1: q_chunks(hh))
                    addf(9, lambda hh=h + 1: k_pack(hh))
                    addf(11, lambda hh=h + 1: q_pack(hh))
                # heads 0-6 projection during the last head's attention
                if h == 7:
                    for i in range(12):
                        ct, qc = i // 2, i % 2
                        addf(i + 1, lambda c=ct, q=qc: proj06(c, q))
                return f

            for h in range(H):
                attn_head(h, mk_fillers(h))

            # ---- tail: head-7 projection + combine + out ----
            for ct in range(CT):
                for qc in range(NQC):
                    proj7(ct, qc)

    nc.compile()
    return nc


def _get_nc():
    if "nc" not in _CACHE:
        _CACHE["nc"] = _build()
    return _CACHE["nc"]


def _prep_shards(x, w_qkv, w_proj, b_proj):
    bf16 = ml_dtypes.bfloat16
    x = np.asarray(x, dtype=np.float32)
    w_qkv = np.asarray(w_qkv, dtype=np.float32)
    w_proj = np.asarray(w_proj, dtype=np.float32)
    b_proj = np.asarray(b_proj, dtype=np.float32)

    wq_b = np.ascontiguousarray(w_qkv[0:DIM].T * SCALE).astype(bf16)
    wk_b = np.ascontiguousarray(w_qkv[DIM:2 * DIM].T).astype(bf16)
    wv_b = np.ascontiguousarray(w_qkv[2 * DIM:3 * DIM].T).astype(bf16)
    wp_b = np.ascontiguousarray(w_proj.T.reshape(H, DH, DIM)).astype(bf16)
    bias = np.ascontiguousarray(b_proj.reshape(DIM, 1))

    in_maps = []
    for c in range(NCORES):
        b, half = divmod(c, 2)
        xt = x[b].T  # [768, 2048]
        if half == 1:
            xt = np.concatenate([xt[:, NQ:], xt[:, :NQ]], axis=1)
        in_maps.append({
            "xt": np.ascontiguousarray(xt).astype(bf16),
            "wq": wq_b,
            "wk": wk_b,
            "wv": wv_b,
            "wp": wp_b,
            "bias": bias,
        })
    return in_maps


def kernel(x, w_qkv, w_proj, b_proj):
    from concourse.bass_utils import run_bass_kernel_spmd

    nc = _get_nc()
    in_maps = _prep_shards(x, w_qkv, w_proj, b_proj)
    res = run_bass_kernel_spmd(nc, in_maps, core_ids=list(range(NCORES)))
    out = np.empty((B, N, DIM), np.float32)
    for c in range(NCORES):
        b, half = divmod(c, 2)
        yT = np.asarray(res.results[c]["out"], dtype=np.float32)  # [768, 1024]
        out[b, half * NQ:(half + 1) * NQ, :] = yT.T
    return out
